# revision 2
# baseline (speedup 1.0000x reference)
"""Trainium2 Bass kernel for nn_Attention (b=4, n=2048, d=1024, 16 heads x 64).

V6: big-exp-op + depth-2 scores ring.

HW facts driving this design (measured via stage ablation + variant sweep):
  - ACT exp has ~0.5-1.5us PER-OP overhead: 128x2048-elem exps beat
    256x1024 by 185us, which beat 512x512 by 127us.
  - A depth-1 scores psum ring serializes scores(j+1) behind exp(j)
    (~210us), so the ring must be >= 2 tiles deep.
  - PSUM is 8 banks: ring 2 x [128,3,512] (6) + av 2 x [65,512] (2).
    exp ops are 1536 elems (1024 for the it-boundary remainder tile).
  - Matmuls pay ~150ns per unique K=128 stationary load; B pairs its
    moving chunks per stationary; stage E runs bf16 (error budget ok).
  - The XBAR DMA-transpose for x^T costs ~97us serialized; PE
    is_transpose matmuls (~26us, overlapped with the x load) win.

Scores stream as a flat group sequence g=(jsp,h01,half) per (hp,it),
packed 3 groups per psum tile; AV consumes exp tiles with a 2-tile lag;
B(hp+1) units interleave into D(hp) as ring-borrowing fillers.

Sharding: 8 cores = 4 batches x 2 head-groups; host sums partials.
"""
import sys

sys.path.insert(0, "/opt/trn_rl_repo")

import ml_dtypes
import numpy as np

import concourse.bass as bass
import concourse.mybir as mybir
import concourse.tile as tile
from concourse import bacc
from concourse.bass import ts, ds

F32 = mybir.dt.float32
BF16 = mybir.dt.bfloat16
FP16 = mybir.dt.float16
AF = mybir.ActivationFunctionType

SEQ = 2048
DIM = 1024
H = 8
HD = 64
QK = 1024
VC = 512
E = 1024
KSUB = DIM // 128  # 8
ITILE = 512
NIT = SEQ // ITILE  # 4
NJS = SEQ // 128  # 16
NHP = H // 2  # 4
NGRP = 32  # score groups per (hp, it): jsp(8) x h01(2) x half(2)
GPT = 3  # groups per psum tile


def build_attention(iters: int = 1, stages: int = 5):
    nc = bacc.Bacc("TRN2", target_bir_lowering=False, debug=False)
    x = nc.dram_tensor("x", [SEQ, DIM], FP16, kind="ExternalInput")
    w_qk = nc.dram_tensor("w_qk", [DIM, QK], FP16, kind="ExternalInput")
    w_v = nc.dram_tensor("w_v", [DIM, VC], FP16, kind="ExternalInput")
    w_proj = nc.dram_tensor("w_proj", [VC, E], BF16, kind="ExternalInput")
    bias = nc.dram_tensor("bias", [E], F32, kind="ExternalInput")
    out = nc.dram_tensor("out", [SEQ, E], F32, kind="ExternalOutput")

    w_qk_r = w_qk.rearrange("(ko p) c -> p ko c", p=128)
    w_v_r = w_v.rearrange("(ko p) c -> p ko c", p=128)
    w_proj_r = w_proj.rearrange("(cs p) e -> p cs e", p=128)

    with tile.TileContext(nc) as tc:
        with (
            tc.tile_pool(name="cpool", bufs=1) as cpool,
            tc.tile_pool(name="xpool", bufs=2) as xpool,
            tc.tile_pool(name="qkring", bufs=4) as qkring,
            tc.tile_pool(name="epool", bufs=4) as epool,
            tc.tile_pool(name="npool", bufs=2) as npool,
            tc.tile_pool(name="opool", bufs=3) as opool,
            tc.tile_pool(name="ps_sp", bufs=2, space="PSUM") as ps_sp,
            tc.tile_pool(name="ps_av", bufs=2, space="PSUM") as ps_av,
        ):
            pools = (cpool, xpool, qkring, epool, npool, opool, ps_sp, ps_av)
            if iters == 1:
                one_iter(tc, nc, x, w_qk_r, w_v_r, w_proj_r, bias, out, pools, stages)
            else:
                with tc.For_i(0, iters, 1):
                    one_iter(
                        tc, nc, x, w_qk_r, w_v_r, w_proj_r, bias, out, pools, stages
                    )
    nc.compile()
    return nc


def one_iter(tc, nc, x, w_qk_r, w_v_r, w_proj_r, bias, out, pools, stages=5):
    cpool, xpool, qkring, epool, npool, opool, ps_sp, ps_av = pools

    # ---- upfront DMAs: critical path (id, x) on sync; weights on scalar ----
    out_r = out.rearrange("(p a) e -> p (a e)", p=128)
    x_r = x.rearrange("(ib sc p) d -> p ib sc d", p=128, sc=4)

    id_dram = nc.inline_tensor(
        np.eye(128, dtype=np.float16 if FP16 == mybir.dt.float16 else ml_dtypes.bfloat16),
        "id128",
    )
    id_sb = cpool.tile([128, 128], FP16, tag="id")
    nc.sync.dma_start(id_sb[:], id_dram.ap())

    x_chunks = []
    for ib in range(4):
        xc = xpool.tile([128, 4, DIM], FP16, tag="xc", name=f"xc{ib}")
        nc.sync.dma_start(xc[:], x_r[:, ib])
        x_chunks.append(xc)

    v_sb = cpool.tile([128, NJS, H * (HD + 1)], FP16, tag="v")
    v_view = v_sb[:].rearrange("p j (h c) -> p j h c", c=HD + 1)
    ones_dram = nc.inline_tensor(
        np.ones(
            (NJS, H * (HD + 1)),
            np.float16 if FP16 == mybir.dt.float16 else ml_dtypes.bfloat16,
        ),
        "ones_fill",
    )
    nc.scalar.dma_start(
        v_sb[:], ones_dram.ap()[None, :, :].to_broadcast((128, NJS, H * (HD + 1)))
    )
    avT = cpool.tile([128, NHP, SEQ], BF16, tag="avT")
    xT = cpool.tile([128, KSUB, SEQ], FP16, tag="xT")
    w_v_sb = cpool.tile([128, KSUB, VC], FP16, tag="wv")
    nc.scalar.dma_start(w_v_sb[:], w_v_r[:])
    wqk_sb = cpool.tile([128, QK // 128, KSUB, 128], FP16, tag="wqk")
    for ct in range(QK // 128):
        nc.scalar.dma_start(wqk_sb[:, ct], w_qk_r[:, :, ts(ct, 128)])
    wproj_sb = cpool.tile([128, VC // 128, E], BF16, tag="wproj")
    nc.scalar.dma_start(wproj_sb[:], w_proj_r[:])
    bias_rep = cpool.tile([128, E], F32, tag="bias")
    nc.scalar.dma_start(bias_rep[:], bias[None, :].to_broadcast((128, E)))

    # ---- Stage A: straight x load + PE transpose into xT ----
    for ib in range(4):
        xc = x_chunks[ib]
        for dc in range(KSUB):
            pt = ps_sp.tile([128, 4, 128], FP16, tag="s", name="pt")
            for sc in range(4):
                nc.tensor.transpose(pt[:, sc, :], xc[:, sc, ts(dc, 128)], id_sb[:])
            nc.vector.tensor_copy(
                xT[:, dc, ds(ib * 512, 512)],
                pt[:].rearrange("p s c -> p (s c)"),
            )

    if stages <= 1:
        nc.sync.dma_start(
            out_r[:].bitcast(FP16)[:, 0 : KSUB * SEQ],
            xT[:].rearrange("p k s -> p (k s)"),
        )
        return

    # ---- Stage C: v = x @ w_v ----
    for jt2 in range(NJS // 2):
        s = ps_sp.tile([128, 2, VC], F32, tag="s", name="psv")
        for ksv in range(KSUB):
            for i in range(2):
                nc.tensor.matmul(
                    s[:, i, :],
                    xT[:, ksv, ts(2 * jt2 + i, 128)],
                    w_v_sb[:, ksv, :],
                    start=(ksv == 0),
                    stop=(ksv == KSUB - 1),
                )
        for i in range(2):
            nc.vector.tensor_copy(
                v_view[:, 2 * jt2 + i, :, 0:HD],
                s[:, i, :].rearrange("p (h c) -> p h c", c=HD),
            )

    if stages <= 2:
        nc.sync.dma_start(
            out_r[:].bitcast(FP16)[:, 0 : NJS * H * (HD + 1)],
            v_sb[:].rearrange("p j c -> p (j c)"),
        )
        return

    # ---- Stage B: q^T/k^T per head-pair (generator of ring-borrowing units) ----
    qk_tiles = {}

    def emit_B(hp):
        """Yield after each (dest, it2) unit: 16 matmuls + 2 copies."""
        qTh = qkring.tile([128, SEQ], FP16, tag="qT", name=f"qT{hp}")
        kTh = qkring.tile([128, SEQ], FP16, tag="kT", name=f"kT{hp}")
        qk_tiles[hp] = (qTh, kTh)
        for ct, dest in ((hp, qTh), (hp + 4, kTh)):
            w_t = wqk_sb[:, ct]
            for it2 in range(NIT // 2):
                s = ps_sp.tile([128, 2, ITILE], F32, tag="s", name="psb")
                for ksv in range(KSUB):
                    for i in range(2):
                        nc.tensor.matmul(
                            s[:, i, :],
                            w_t[:, ksv, :],
                            xT[:, ksv, ts(2 * it2 + i, ITILE)],
                            start=(ksv == 0),
                            stop=(ksv == KSUB - 1),
                        )
                for i in range(2):
                    nc.vector.tensor_copy(dest[:, ts(2 * it2 + i, ITILE)], s[:, i, :])
                yield

    def run_all(gen):
        if gen is not None:
            for _ in gen:
                pass

    run_all(emit_B(0))

    if stages <= 3:
        for hp in range(1, NHP):
            run_all(emit_B(hp))
        for hp in range(NHP):
            qTh, kTh = qk_tiles[hp]
            nc.sync.dma_start(
                out_r[:, ds(hp * 4096, SEQ)].bitcast(FP16)[:, 0:SEQ], qTh[:]
            )
            nc.sync.dma_start(
                out_r[:, ds(hp * 4096 + SEQ, SEQ)].bitcast(FP16)[:, 0:SEQ], kTh[:]
            )
        return

    # ---- Stage E (generator; per-(it128) units) ----
    def emit_E(it4):
        for it in range(4 * it4, 4 * it4 + 4):
            s = ps_sp.tile([128, 2, ITILE], F32, tag="s", name="pse")
            for cs in range(VC // 128):
                for et in range(2):
                    nc.tensor.matmul(
                        s[:, et, :],
                        avT[:, cs, ts(it, 128)],
                        wproj_sb[:, cs, ts(et, ITILE)],
                        start=(cs == 0),
                        stop=(cs == VC // 128 - 1),
                    )
            for et in range(2):
                o = opool.tile([128, ITILE], F32, tag="o")
                nc.vector.tensor_add(o[:], s[:, et, :], bias_rep[:, ts(et, ITILE)])
                nc.sync.dma_start(out[ts(it, 128), ts(et, ITILE)], o[:])
            yield

    # ---- Stage D: flat group stream, 3 groups per psum tile ----
    # group g = (jsp, h01, half): jsp = g//4, h01 = (g//2)%2, half = g%2
    def emit_D(hp, filler=None, fillers_by_it=None):
        qTh, kTh = qk_tiles[hp]
        cur = [filler]

        def fill():
            f = cur[0]
            if f is not None:
                try:
                    next(f)
                except StopIteration:
                    cur[0] = None

        ntile = (NGRP + GPT - 1) // GPT  # 11 tiles: 10x3 + 1x2
        for it in range(NIT):
            if fillers_by_it is not None:
                run_all(cur[0])  # drain any unfinished filler first
                cur[0] = fillers_by_it(it)
            av_ps = [
                ps_av.tile([HD + 1, ITILE], F32, tag="av", name=f"av{h01}")
                for h01 in range(2)
            ]

            def emit_av(tidx, e):
                for k in range(min(GPT, NGRP - tidx * GPT)):
                    g = tidx * GPT + k
                    jsp, h01, half = g // 4, (g // 2) % 2, g % 2
                    nc.tensor.matmul(
                        av_ps[h01][:],
                        v_view[:, 2 * jsp + half, 2 * hp + h01, :],
                        e[:, k, :],
                        start=(g <= 1),  # g=0 (h01=0) and g=1? no: see below
                        stop=(g >= NGRP - 2),
                    )

            # start/stop per h01 accumulator: h01=0 first at g=0, h01=1 first
            # at g=2; h01=0 last at g=29, h01=1 last at g=31 -- handled via
            # explicit group math instead of the g<=1 shortcut above.
            def emit_av(tidx, e):  # noqa: F811
                for k in range(min(GPT, NGRP - tidx * GPT)):
                    g = tidx * GPT + k
                    jsp, h01, half = g // 4, (g // 2) % 2, g % 2
                    first = jsp == 0 and half == 0
                    last = jsp == NJS // 2 - 1 and half == 1
                    nc.tensor.matmul(
                        av_ps[h01][:],
                        v_view[:, 2 * jsp + half, 2 * hp + h01, :],
                        e[:, k, :],
                        start=first,
                        stop=last,
                    )

            pend = []
            for tidx in range(ntile):
                g0 = tidx * GPT
                gn = min(GPT, NGRP - g0)
                sp = ps_sp.tile([128, GPT, ITILE], F32, tag="s", name="sc")
                e = epool.tile([128, GPT, ITILE], BF16, tag="e")
                for k in range(gn):
                    g = g0 + k
                    jsp, h01, half = g // 4, (g // 2) % 2, g % 2
                    sl = slice(h01 * 64, h01 * 64 + 64)
                    nc.tensor.matmul(
                        sp[:, k, :],
                        kTh[sl, ts(2 * jsp + half, 128)],
                        qTh[sl, ts(it, ITILE)],
                        start=True,
                        stop=True,
                    )
                nc.scalar.activation(e[:, 0:gn, :], sp[:, 0:gn, :], AF.Exp)
                pend.append((tidx, e))
                if tidx % 4 == 3:
                    fill()
                if len(pend) > 2:
                    t0, e0 = pend.pop(0)
                    emit_av(t0, e0)
            for t0, e0 in pend:
                emit_av(t0, e0)
            if it == NIT - 1:
                run_all(cur[0])  # drain leftover filler units

            for h01 in range(2):
                avU = npool.tile([HD + 1, ITILE], F32, tag="avU")
                nc.vector.tensor_copy(avU[:], av_ps[h01][:])
                rc = npool.tile([1, ITILE], F32, tag="rc")
                nc.vector.reciprocal(rc[:], avU[HD : HD + 1, :])
                rr = npool.tile([64, ITILE], F32, tag="rr")
                nc.gpsimd.partition_broadcast(rr[:], rc[:])
                if h01 == 0:
                    nc.vector.tensor_mul(
                        avT[0:64, hp, ts(it, ITILE)], avU[0:HD, :], rr[:]
                    )
                else:
                    tmp = npool.tile([64, ITILE], BF16, tag="tmp")
                    nc.vector.tensor_mul(tmp[:], avU[0:HD, :], rr[:])
                    nc.sync.dma_start(avT[64:128, hp, ts(it, ITILE)], tmp[:])

    for hp in range(NHP - 1):
        emit_D(hp, emit_B(hp + 1))

    if stages <= 4:
        emit_D(NHP - 1)
        nc.sync.dma_start(
            out_r[:, 0 : NHP * SEQ // 2],
            avT[:].rearrange("p k s -> p (k s)").bitcast(F32),
        )
        return

    emit_D(NHP - 1)
    for it4 in range(NIT):
        run_all(emit_E(it4))


# ---------------- host side ----------------

_CACHE = {}


def _get_runner():
    if "runner" not in _CACHE:
        import jax
        from jax.sharding import Mesh, PartitionSpec
        from jax.experimental.shard_map import shard_map
        from concourse import bass2jax

        nc = build_attention(iters=1)
        bass2jax.install_neuronx_cc_hook()

        in_names, out_names, out_avals, zero_shapes = [], [], [], []
        partition_name = nc.partition_id_tensor.name if nc.partition_id_tensor else None
        for alloc in nc.m.functions[0].allocations:
            if not isinstance(alloc, mybir.MemoryLocationSet):
                continue
            name = alloc.memorylocations[0].name
            if alloc.kind == "ExternalInput":
                if name != partition_name:
                    in_names.append(name)
            elif alloc.kind == "ExternalOutput":
                out_names.append(name)
                shape = tuple(alloc.tensor_shape)
                dtype = mybir.dt.np(alloc.dtype)
                out_avals.append(jax.core.ShapedArray(shape, dtype))
                zero_shapes.append((shape, dtype))
        n_params = len(in_names)
        n_outs = len(out_avals)
        all_names = in_names + out_names
        if partition_name is not None:
            all_names = all_names + [partition_name]
        donate = tuple(range(n_params, n_params + n_outs))

        def _body(*args):
            operands = list(args)
            if partition_name is not None:
                operands.append(bass2jax.partition_id_tensor())
            outs = bass2jax._bass_exec_p.bind(
                *operands,
                out_avals=tuple(out_avals),
                in_names=tuple(all_names),
                out_names=tuple(out_names),
                lowering_input_output_aliases=(),
                sim_require_finite=True,
                sim_require_nnan=True,
                nc=nc,
            )
            return tuple(outs)

        devices = jax.devices()[:8]
        mesh = Mesh(np.asarray(devices), ("core",))
        in_specs = (PartitionSpec("core"),) * (n_params + n_outs)
        out_specs = (PartitionSpec("core"),) * n_outs
        sharded = jax.jit(
            shard_map(
                _body,
                mesh=mesh,
                in_specs=in_specs,
                out_specs=out_specs,
                check_rep=False,
            ),
            donate_argnums=donate,
            keep_unused=True,
        )
        _CACHE["runner"] = (sharded, in_names, out_names, out_avals, zero_shapes)
    return _CACHE["runner"]


def _shard_inputs(x, w_qkv, w_proj, b_proj):
    """Per-core input dicts. Core c: batch c//2, head-group c%2."""
    SCALE = HD**-0.5
    bf16 = np.float16
    in_maps = []
    zeros_bias = np.zeros_like(b_proj)
    for c in range(8):
        b = c // 2
        hg = c % 2
        qs = slice(hg * 512, (hg + 1) * 512)
        ks = slice(1024 + hg * 512, 1024 + (hg + 1) * 512)
        vs = slice(2048 + hg * 512, 2048 + (hg + 1) * 512)
        w_qk_c = np.concatenate(
            [w_qkv[:, qs] * np.float32(SCALE), w_qkv[:, ks]], axis=1
        ).astype(bf16)
        in_maps.append(
            {
                "x": x[b].astype(bf16),
                "w_qk": w_qk_c,
                "w_v": w_qkv[:, vs].astype(bf16),
                "w_proj": np.ascontiguousarray(
                    w_proj[hg * 512 : (hg + 1) * 512]
                ).astype(ml_dtypes.bfloat16),
                "bias": b_proj if hg == 0 else zeros_bias,
            }
        )
    return in_maps


def kernel(x, w_qkv, w_proj, b_proj):
    import jax
    import jax.numpy as jnp

    x = np.asarray(x, dtype=np.float32)
    w_qkv = np.asarray(w_qkv, dtype=np.float32)
    w_proj = np.asarray(w_proj, dtype=np.float32)
    b_proj = np.asarray(b_proj, dtype=np.float32)

    sharded, in_names, out_names, out_avals, zero_shapes = _get_runner()
    in_maps = _shard_inputs(x, w_qkv, w_proj, b_proj)
    concat_in = [
        np.concatenate([in_maps[c][name] for c in range(8)], axis=0)
        for name in in_names
    ]
    zeros = [jnp.zeros((8 * s[0], *s[1:]), dt) for (s, dt) in zero_shapes]
    outs = sharded(*concat_in, *zeros)
    out_np = np.asarray(outs[out_names.index("out")]).reshape(8, SEQ, E)
    full = np.empty((4, SEQ, E), dtype=np.float32)
    for b in range(4):
        full[b] = out_np[2 * b] + out_np[2 * b + 1]
    return full
